# revision 46
# baseline (speedup 1.0000x reference)
"""Multi-head causal attention (B=64, T=256, C=1024, H=16) on 8 TRN2 NeuronCores.

Data-parallel over batch (8 batches = 2048 tokens per core). V/out projections
and attention in bf16 (fp32 PSUM); the QK projection runs in fp8-e4m3
DoubleRow (2 MACs/cell/cycle, contraction 256 per matmul) — HW-measured at
the same 216ns issue spacing as a bf16 K=128 matmul, i.e. a true 2x.
~258 us HW vs the 318 us all-bf16 predecessor; output rel err 1.55e-2
(gate 2e-2, bf16 baseline was 3.7e-3).

Design notes:
  - QK fp8: x is quantized e4m3 at x*16 (host side, a second copy of x; the
    bf16 copy still feeds the V projection so V/attention accuracy is
    unaffected). W_q|W_k are quantized with a per-feature-column scale
    alpha_f = 120/max|col| to stay in e4m3's normal range (naive folding of
    1/sqrt(D) lands half the weights in fp8 subnormals). The DVE epilogue
    applies (psum * s_f + b_f) per feature partition via tensor_scalar,
    un-doing SX*alpha_f and folding the softmax scale + bias.
  - DoubleRow operands are 3D APs [128, 2, N]: plane i carries contraction
    rows 256*c2 + 128*i + p. Moving FD=512 (1024 fp8) is the ISA max; the
    256-col DR LDWEIGHTS (~135ns) hides under the chain.
  - Merged schedule: attention for batch b starts as soon as QK quarter b//2
    is done. Quarter 0 + 8 tiles of quarter 1 run in the prelude interleaved
    with V(0)/V(1); the rest drains one tile per attention pair through
    batches 0-5. This spreads the ACT/DVE softmax epilogue (~3.8us/pair)
    over the whole kernel span instead of a back-loaded attention phase.
  - Startup: input layouts are partition-major so every DMA piece is
    contiguous per partition — DMA enqueue time scales with descriptor
    count (a 1024-run strided piece measured 12us of engine time vs ~0.7us
    for 128 runs, stalling the issuing engine's FIFO and the PE). Pieces
    are split along consumption order over the sync/scalar/gpsimd rings
    (sync's ring measures ~62GB/s, the others ~125GB/s), late inputs are
    emitted mid-stream, and 20 warmup matmuls on a memset scratch region
    cover the ~7us framework preamble + DMA lead-in so the PE HAM
    clock-gate is at 8/8 when real work arrives.
  - Tail: the last batch's out-projection uses 4 PSUM banks (2 from ps_proj
    + 2 borrowed from ps_st, which is idle after the last s1): each chunk's
    ct 0..6 stream is emitted around the final s2 and the ct=7 matmul +
    bias-add + split DMAs of chunk i ride behind chunk i+1's stream, so
    output DMAs drain ~6us before the last matmul.
  - Attention per head pair (even head PSUM bank 0, odd bank 1): scores via
    2 matmuls over the causal block structure (s1 heads run concurrently in
    PE row groups 0:64/64:128 via auto tile_position); triangular mask added
    in PSUM by one DVE add per pair; exp on ACT; softmax denominator
    replicated across 64 partitions via v_aug ones columns so the
    reciprocal is one partition-parallel DVE op; normalize muls on GpSimd
    (no PSUM port, SBUF-only work). s2 runs both FD256 matmuls before the
    FD128s so the va LDWEIGHTS hides under the longer fills.
  - PSUM start=True resets the whole 2KB bank: only the first matmul of a
    chain sets it.
"""

import sys
import time

sys.path.insert(0, "/opt/trn_rl_repo")

import numpy as np
import ml_dtypes

BF16 = ml_dtypes.bfloat16
F8E4 = ml_dtypes.float8_e4m3

EMBED = 1024
HEADS = 16
DHEAD = 64
T = 256
B = 64
NCORES = 8
BLOC = B // NCORES          # 8 batches per core
TOK = BLOC * T              # 2048 tokens per core
CT = EMBED // 128           # 8 bf16 contraction tiles
C2 = 4                      # fp8 DoubleRow contraction tiles (256 each)
SX = 16.0                   # x fp8 pre-scale
ALPHA_TGT = 120.0           # per-column weight max after scaling

_CACHE = {}


def _build():
    import concourse.bacc as bacc
    import concourse.mybir as mybir
    import concourse.tile as tile

    f32 = mybir.dt.float32
    bf16 = mybir.dt.bfloat16
    f8 = mybir.dt.float8e4
    EXP = mybir.ActivationFunctionType.Exp
    DR = mybir.MatmulPerfMode.DoubleRow
    MUL = mybir.AluOpType.mult
    ADD = mybir.AluOpType.add

    nc = bacc.Bacc("TRN2", target_bir_lowering=False, debug=False)

    # all layouts are partition-major with whole DMA pieces contiguous per
    # partition: DMA enqueue (descriptor-generation) time scales with the
    # run count, and a 1024-run strided enqueue was measured at 12us of
    # engine time (vs ~0.7us for 128 contiguous runs)
    xt_d = nc.dram_tensor("xt", [128, 16, CT, 128], bf16, kind="ExternalInput").ap()
    x8_d = nc.dram_tensor("x8", [128, 4, C2, 2, 512], f8, kind="ExternalInput").ap()
    w8_d = nc.dram_tensor("w8", [128, 4, C2, 2, 512], f8, kind="ExternalInput").ap()
    wv_d = nc.dram_tensor("wv", [128, 2, 2, 4, 512], bf16, kind="ExternalInput").ap()
    wo_d = nc.dram_tensor("wo", [128, 2, 4, EMBED], bf16, kind="ExternalInput").ap()
    sqk_d = nc.dram_tensor("sqk", [128, 16, 2], f32, kind="ExternalInput").ap()
    bo2_d = nc.dram_tensor("bo2", [128, EMBED], bf16, kind="ExternalInput").ap()
    mtri_d = nc.dram_tensor("mtri", [128, 2, 2, 128], f32, kind="ExternalInput").ap()
    out_d = nc.dram_tensor("out", [TOK, EMBED], f32, kind="ExternalOutput").ap()

    with tile.TileContext(nc) as tc:
        with (
            tc.tile_pool(name="const", bufs=1) as constp,
            tc.tile_pool(name="expp", bufs=3) as expp,
            tc.tile_pool(name="recipp", bufs=2) as recipp,
            tc.tile_pool(name="ycpp", bufs=2) as ycpp,
            tc.tile_pool(name="ytp", bufs=2) as ytp,
            tc.tile_pool(name="outp", bufs=2) as outp,
            tc.tile_pool(name="ps_proj", bufs=2, space="PSUM") as ps_proj,
            tc.tile_pool(name="ps_st", bufs=2, space="PSUM") as ps_st,
            tc.tile_pool(name="ps_yt", bufs=2, space="PSUM") as ps_yt,
        ):
            xt_s = constp.tile([128, 16, CT, 128], bf16)   # [p, tpiece, ct, t']
            x8_s = constp.tile([128, 4, C2, 2, 512], f8)
            w8_s = constp.tile([128, 4, C2, 2, 512], f8)
            wv_s = constp.tile([128, 2, 2, 4, 512], bf16)  # [p, fjh, ch, ct', f']
            wo_s = constp.tile([128, 2, 4, EMBED], bf16)   # [p, ch, ct', f]
            sqk_s = constp.tile([128, 16, 2], f32)
            bo2_s = constp.tile([128, EMBED], bf16)
            mtri4_s = constp.tile([128, 2, 2, 128], f32)
            NVBUF = 2
            v_aug = constp.tile([128, NVBUF, 2, 16 * 128], bf16)
            ones_view = v_aug.rearrange("p u tt (h x) -> p u tt h x", x=128)
            qk_s = constp.tile([128, 16, TOK], bf16)

            # ---- PE warmup: memset a scratch region of qk_s (overwritten by
            # the QK epilogue much later), then 8 matmuls so HAM reaches 8/8
            # during the DMA lead-in ----
            junk = qk_s[:, 0, 0:640]
            nc.gpsimd.memset(junk, 1.0)
            wscr = ps_yt.tile([128, 2, T], f32, tag="yt_ps")
            wview = wscr.rearrange("p a b -> p (a b)")
            for i in range(20):
                nc.tensor.matmul(
                    wview[:, 0:512], junk[:, 0:128], junk[:, 128:640],
                    start=True, stop=True, skip_group_check=(i > 0),
                )

            # ---- input DMAs, split along consumption order, spread over
            # 4 queues (dependency granularity is per DMA instruction) ----

            # Upfront: ONLY the prelude's critical path (~5.3MB), all pieces
            # contiguous per partition. Everything else is emitted later in
            # the stream (staged below) so late transfers can't stall the
            # startup enqueues via ring/semaphore pressure.
            # Consumption order: warmup -> qk f0-3 -> V0(f0,tt0) ->
            # qk f4-7 -> V0(f0,tt1) -> qk f8-11 -> V0(f1,tt0) ->
            # qk f12-15 -> V0(f1,tt1) -> V1/qk-q1
            # (sync's DMA ring measures ~62GB/s vs ~125GB/s for the scalar/
            # gpsimd rings — keep sync's upfront load light)
            nc.sync.dma_start(out=sqk_s[:], in_=sqk_d[:])
            nc.sync.dma_start(out=xt_s[:, 0], in_=xt_d[:, 0])
            nc.sync.dma_start(out=xt_s[:, 1], in_=xt_d[:, 1])
            nc.sync.dma_start(out=w8_s[:, 3], in_=w8_d[:, 3])
            nc.sync.dma_start(out=xt_s[:, 2], in_=xt_d[:, 2])
            nc.sync.dma_start(out=xt_s[:, 3], in_=xt_d[:, 3])

            nc.scalar.dma_start(out=x8_s[:, 0], in_=x8_d[:, 0])
            nc.scalar.dma_start(out=wv_s[:, 0, 0], in_=wv_d[:, 0, 0])
            nc.scalar.dma_start(out=w8_s[:, 2], in_=w8_d[:, 2])
            nc.scalar.dma_start(out=wv_s[:, 1, 0], in_=wv_d[:, 1, 0])

            nc.gpsimd.dma_start(out=w8_s[:, 0], in_=w8_d[:, 0])
            nc.gpsimd.dma_start(out=wv_s[:, 0, 1], in_=wv_d[:, 0, 1])
            nc.gpsimd.memset(ones_view[:, 0, :, :, 0:64], 1.0)
            nc.gpsimd.dma_start(out=w8_s[:, 1], in_=w8_d[:, 1])
            nc.gpsimd.memset(ones_view[:, 1, :, :, 0:64], 1.0)
            nc.gpsimd.dma_start(out=wv_s[:, 1, 1], in_=wv_d[:, 1, 1])

            def xt_tok(ct, lo, n):
                """AP for xt tokens [lo, lo+n) of contraction chunk ct."""
                assert n == 128 and lo % 128 == 0
                return xt_s[:, lo // 128, ct, :]

            # ---------------- emission helpers ----------------

            def v_gen(b):
                """V projection of batch b (token-major into v_aug[b%2]).
                9 yields per chunk (chunk-atomic drains keep <=2 ps_proj
                tiles in flight)."""
                for fjh in range(2):
                    for tt in range(2):
                        ps = ps_proj.tile([128, 512], f32, tag="ps_proj")
                        tbase = b * T + tt * 128
                        for ct in range(CT):
                            nc.tensor.matmul(
                                ps[:],
                                xt_tok(ct, tbase, 128),
                                wv_s[:, fjh, ct // 4, ct % 4, :],
                                start=(ct == 0),
                                stop=(ct == CT - 1),
                            )
                            yield
                        dst = v_aug.rearrange("p u tt (h x) -> p u tt h x", x=128)[
                            :, b % NVBUF, tt, fjh * 8:(fjh + 1) * 8, 64:128
                        ]
                        nc.scalar.copy(
                            out=dst, in_=ps.rearrange("p (h x) -> p h x", x=64)
                        )
                        yield

            def qk_gen():
                """QK projection, fp8 DoubleRow, quarter-major. 5 yields
                per [128 feat x 512 tok] tile (4 MM + 1 DVE epilogue)."""
                for q in range(4):
                    for f in range(16):
                        piece, fo = divmod(f, 4)
                        ps = ps_proj.tile([128, 512], f32, tag="ps_proj")
                        for c2 in range(C2):
                            nc.tensor.matmul(
                                ps[:],
                                w8_s[:, piece, c2, :, fo * 128:(fo + 1) * 128],
                                x8_s[:, q, c2, :, :],
                                start=(c2 == 0),
                                stop=(c2 == C2 - 1),
                                perf_mode=DR,
                            )
                            yield
                        nc.vector.tensor_scalar(
                            out=qk_s[:, f, q * 512:(q + 1) * 512],
                            in0=ps[:],
                            scalar1=sqk_s[:, f, 0:1],
                            scalar2=sqk_s[:, f, 1:2],
                            op0=MUL,
                            op1=ADD,
                        )
                        yield

            def out_gen(b):
                """Output projection of batch b from yt tile. 9 yields/chunk."""
                yt = yt_tiles[b]
                engs = [nc.sync, nc.gpsimd]
                for tt in range(2):
                    for fjh in range(2):
                        ps = ps_proj.tile([128, 512], f32, tag="ps_proj")
                        tbase = b * T + tt * 128
                        for ct in range(CT):
                            nc.tensor.matmul(
                                ps[:],
                                yt[:, ct, tt * 128:(tt + 1) * 128],
                                wo_s[:, ct // 4, ct % 4,
                                     fjh * 512:(fjh + 1) * 512],
                                start=(ct == 0),
                                stop=(ct == CT - 1),
                            )
                            yield
                        o_sb = outp.tile([128, 512], f32, tag="o_sb")
                        nc.vector.tensor_add(
                            out=o_sb[:], in0=ps[:],
                            in1=bo2_s[:, fjh * 512:(fjh + 1) * 512],
                        )
                        engs[(2 * tt + fjh) % 2].dma_start(
                            out=out_d[tbase:tbase + 128,
                                      fjh * 512:(fjh + 1) * 512],
                            in_=o_sb[:],
                        )
                        yield

            def drain(gen, n):
                if gen is None:
                    return
                for _ in range(n):
                    if next(gen, StopIteration) is StopIteration:
                        return

            def attn_s1_pair(b, f):
                """Scores+exp for head pair (2f, 2f+1) of batch b.

                sT layout [128, hh, slot, 128]: per pair-half hh (even head in
                PSUM bank 0, odd in bank 1), slots = A (s-diag t0:128),
                B (s-diag t128:256), C (s0:128 strip t128:256), pad. The two
                hh halves run concurrently in PE row groups 0:64 / 64:128
                (tile_position auto-derived from base partitions).
                """
                t0 = b * T
                fq, fk = f, 8 + f
                sT = ps_st.tile([128, 2, 4, 128], f32, tag="sT")
                for hh in range(2):
                    po = hh * 64
                    qT = qk_s[po:po + 64, fq, t0:t0 + T]
                    kT = qk_s[po:po + 64, fk, t0:t0 + T]
                    nc.tensor.matmul(
                        sT[:, hh, 0::2, :], kT[:, 0:128], qT, start=True, stop=True
                    )
                    nc.tensor.matmul(
                        sT[:, hh, 1, :], kT[:, 128:256], qT[:, 128:256],
                        start=False, stop=True, skip_group_check=True,
                    )
                # triangular additive mask on all four diagonal blocks at once
                nc.vector.tensor_add(
                    out=sT[:, :, 0:2, :], in0=sT[:, :, 0:2, :], in1=mtri4_s[:]
                )
                expT = expp.tile([128, 2, 4, 128], bf16, tag="expT")
                nc.scalar.activation(
                    out=expT[:, :, 0:3, :], in_=sT[:, :, 0:3, :], func=EXP
                )
                return expT

            def attn_s2_pair(b, f, expT):
                yt = yt_tiles[b]
                va = v_aug[:, b % NVBUF]
                yp = ps_yt.tile([128, 2, T], f32, tag="yt_ps")
                # both FD256 matmuls first, FD128s after: the ~100ns va
                # LDWEIGHTS hides under a 107ns FD256 fill but not under a
                # 53ns FD128 fill, so alternating exposes it twice per pair
                for hh in range(2):
                    h = 2 * f + hh
                    nc.tensor.matmul(
                        yp[:, hh, :], va[:, 0, h * 128:(h + 1) * 128],
                        expT[:, hh, 0::2, :],
                        start=(hh == 0), stop=True, skip_group_check=(hh == 1),
                    )
                for hh in range(2):
                    h = 2 * f + hh
                    nc.tensor.matmul(
                        yp[:, hh, 128:256], va[:, 1, h * 128:(h + 1) * 128],
                        expT[:, hh, 1, :],
                        start=False, stop=True, skip_group_check=True,
                    )
                # rows 0:64 = denominators (replicated), 64:128 = unnormalized y
                rec = recipp.tile([64, 2, T], f32, tag="rec")
                nc.vector.reciprocal_approx_fast(out=rec[:], in_=yp[0:64, :, :])
                ycp = ycpp.tile([64, 2, T], f32, tag="ycp")
                nc.scalar.copy(out=ycp[:], in_=yp[64:128, :, :])
                for hh in range(2):
                    nc.gpsimd.tensor_mul(
                        out=yt[hh * 64:(hh + 1) * 64, f, :],
                        in0=ycp[:, hh, :], in1=rec[:, hh, :],
                    )

            # ---------------- schedule ----------------

            yt_tiles = {}

            # prelude: qk-tile quads and V(0) chunks interleaved in DMA
            # arrival order, then V(1) chunks with qk quarter-1 pairs.
            # Non-startup input DMAs are emitted in stages here, well before
            # their consumers but after the startup transfers.
            qkg = qk_gen()
            vg0 = v_gen(0)
            for i in range(4):
                drain(qkg, 20)
                drain(vg0, 9)
                if i == 0:
                    nc.sync.dma_start(out=mtri4_s[:], in_=mtri_d[:])
                    nc.scalar.dma_start(out=xt_s[:, 4:16], in_=xt_d[:, 4:16])
                    nc.gpsimd.dma_start(out=x8_s[:, 1], in_=x8_d[:, 1])
                if i == 2:
                    nc.sync.dma_start(out=bo2_s[:], in_=bo2_d[:])
                    nc.gpsimd.dma_start(out=x8_s[:, 2], in_=x8_d[:, 2])
                    nc.gpsimd.dma_start(out=wo_s[:, 1], in_=wo_d[:, 1])
            nc.scalar.dma_start(out=wo_s[:, 0], in_=wo_d[:, 0])
            nc.scalar.dma_start(out=x8_s[:, 3], in_=x8_d[:, 3])

            # attention, batch-major; QK quarters 1-3 drain one tile per
            # pair through b0-b5; V(b+1) / out(b-1) interleave as chunks.
            # pend (the s2 of the previous s1 pair) carries across batch
            # boundaries so the PE queue always has s1 work ahead of a
            # dependency-stalled s2.
            # b0 doubles as prelude phase 2: its s1/s2 pairs need only
            # quarter-0 data (resident early), so they keep the PE fed while
            # V(1), quarter-1 tiles, and the late xt/x8 pieces stream in.
            pend = None
            for b in range(BLOC):
                yt_tiles[b] = ytp.tile([128, CT, T], bf16, tag="yt", name=f"yt{b}")
                vg = v_gen(b + 1) if b <= 6 else None
                og = out_gen(b - 1) if b >= 1 else None
                for f in range(8):
                    e = attn_s1_pair(b, f)
                    if pend is not None:
                        attn_s2_pair(*pend)
                    drain(qkg, 10 if b == 0 else 5)
                    if vg is not None and f % 2 == 0:
                        drain(vg, 9)
                    if og is not None and f % 2 == 1:
                        drain(og, 9)
                    pend = (b, f, e)
                drain(vg, 99)
                drain(og, 99)
                if b >= 1:
                    del yt_tiles[b - 1]

            # tail: out-proj of the last batch on 4 PSUM banks (2 ps_proj +
            # 2 borrowed from ps_st, idle after the last s1). ct 0..6 of all
            # chunks surround the final s2 so only 4 ct=7 matmuls + bias
            # adds + split DMAs trail the last normalize.
            yt7 = yt_tiles[BLOC - 1]
            tail_ps = [ps_proj.tile([128, 512], f32, tag="ps_proj",
                                    name=f"tailp{i}")
                       for i in range(2)]
            for i in range(2):
                t = ps_st.tile([128, 2, 4, 128], f32, tag="sT",
                               name=f"tailb{i}")
                tail_ps.append(t.rearrange("p a b c -> p (a b c)")[:, 0:512])
            chunks = [(0, 0), (0, 1), (1, 0), (1, 1)]

            def tail_mm(ci, ct):
                tt, fjh = chunks[ci]
                nc.tensor.matmul(
                    tail_ps[ci][:],
                    yt7[:, ct, tt * 128:(tt + 1) * 128],
                    wo_s[:, ct // 4, ct % 4, fjh * 512:(fjh + 1) * 512],
                    start=(ct == 0), stop=(ct == CT - 1),
                )

            engs = [nc.sync, nc.scalar, nc.gpsimd]
            tb7 = (BLOC - 1) * T

            def tail_epi(ci):
                """ct=7 matmul + bias adds + split DMAs for tail chunk ci.
                The last chunk splits 4 ways so the final transfer drains in
                a quarter of the time."""
                tt, fjh = chunks[ci]
                tail_mm(ci, 7)
                o_sb = outp.tile([128, 512], f32, tag="o_sb",
                                 name=f"tailo{ci}")
                nh = 4 if ci == 3 else 2
                w = 512 // nh
                for hv in range(nh):
                    sl = slice(hv * w, (hv + 1) * w)
                    fl = fjh * 512 + hv * w
                    nc.vector.tensor_add(
                        out=o_sb[:, sl], in0=tail_ps[ci][:, sl],
                        in1=bo2_s[:, fl:fl + w],
                    )
                    engs[(2 * ci + hv) % 3].dma_start(
                        out=out_d[tb7 + tt * 128:tb7 + (tt + 1) * 128,
                                  fl:fl + w],
                        in_=o_sb[:, sl],
                    )

            # stagger: each chunk's ct7+epilogue is emitted behind the next
            # chunk's ct0-6 stream so DMAs start draining ~6us before the
            # last matmul instead of all trailing it
            for ct in range(7):
                tail_mm(0, ct)
            attn_s2_pair(*pend)
            for ct in range(7):
                tail_mm(1, ct)
            tail_epi(0)
            for ct in range(7):
                tail_mm(2, ct)
            tail_epi(1)
            for ct in range(7):
                tail_mm(3, ct)
            tail_epi(2)
            tail_epi(3)

    nc.compile()
    return nc


def _prep_inputs(x, Wqkv, bqkv, Wo, bo):
    """Host-side prep: shard batch, transpose x, fp8-quantize the QK path
    with per-feature-column weight scales, fold scales/biases, bf16."""
    scale = DHEAD ** -0.5
    Wq = Wqkv[:, :EMBED]
    Wk = Wqkv[:, EMBED:2 * EMBED]
    Wv = Wqkv[:, 2 * EMBED:]

    Wqk = np.concatenate([Wq, Wk], axis=1).astype(np.float32)   # [C, 2C]
    alpha = (ALPHA_TGT / np.abs(Wqk).max(axis=0)).astype(np.float32)
    w8 = (Wqk * alpha).astype(F8E4)
    # [k, f] -> [p, piece, c2, i, j]; k = c2*256 + i*128 + p, f = piece*512+j
    w8_f = np.ascontiguousarray(
        w8.reshape(C2, 2, 128, 4, 512).transpose(2, 3, 0, 1, 4)
    )

    mult = np.concatenate(
        [np.full(EMBED, scale, np.float32), np.ones(EMBED, np.float32)]
    ) / (SX * alpha)
    bias = np.concatenate(
        [bqkv[:EMBED] * scale, bqkv[EMBED:2 * EMBED]]
    ).astype(np.float32)
    sqk = np.stack([mult, bias], axis=-1)                        # [2C, 2]
    sqk_f = np.ascontiguousarray(sqk.reshape(16, 128, 2).transpose(1, 0, 2))

    # wv [p, fjh, ch, ct', f'] = Wv[(ch*4+ct')*128 + p, fjh*512 + f']
    wv_f = np.ascontiguousarray(
        Wv.astype(BF16).reshape(2, 4, 128, 2, 512).transpose(2, 3, 0, 1, 4)
    )
    # wo [p, ch, ct', f] = Wo[(ch*4+ct')*128 + p, f]
    wo_f = np.ascontiguousarray(
        Wo.astype(BF16).reshape(2, 4, 128, EMBED).transpose(2, 0, 1, 3)
    )

    bv = bqkv[2 * EMBED:]
    bo2 = (bv.astype(np.float64) @ Wo.astype(np.float64) + bo).astype(np.float32)
    bo2_f = np.ascontiguousarray(np.broadcast_to(bo2, (128, EMBED)).astype(BF16))

    s_idx = np.arange(128)[:, None]
    t_idx = np.arange(128)[None, :]
    mtri = np.where(s_idx <= t_idx, 0.0, -1e9).astype(np.float32)
    mtri = np.ascontiguousarray(
        np.broadcast_to(mtri[:, None, None, :], (128, 2, 2, 128))
    )

    in_maps = []
    for c in range(NCORES):
        xs = x[c * BLOC:(c + 1) * BLOC].reshape(TOK, EMBED)
        xT = np.ascontiguousarray(xs.T)                          # [C, TOK]
        # xt [p, tp, ct, t'] = xT[ct*128 + p, tp*128 + t']
        xt = np.ascontiguousarray(
            xT.astype(BF16).reshape(CT, 128, 16, 128).transpose(1, 2, 0, 3)
        )
        x8 = (xT * SX).astype(F8E4)
        # [k, t] -> [p, q, c2, i, j]; k = c2*256 + i*128 + p, t = q*512 + j
        x8_f = np.ascontiguousarray(
            x8.reshape(C2, 2, 128, 4, 512).transpose(2, 3, 0, 1, 4)
        )
        in_maps.append(
            {
                "xt": xt,
                "x8": x8_f,
                "w8": w8_f,
                "wv": wv_f,
                "wo": wo_f,
                "sqk": sqk_f,
                "bo2": bo2_f,
                "mtri": mtri,
            }
        )
    return in_maps


def kernel(x, Wqkv, bqkv, Wo, bo):
    from concourse.bass_utils import run_bass_kernel_spmd

    x = np.asarray(x, dtype=np.float32)
    Wqkv = np.asarray(Wqkv, dtype=np.float32)
    bqkv = np.asarray(bqkv, dtype=np.float32)
    Wo = np.asarray(Wo, dtype=np.float32)
    bo = np.asarray(bo, dtype=np.float32)

    if "nc" not in _CACHE:
        _CACHE["nc"] = _build()
    nc = _CACHE["nc"]

    in_maps = _prep_inputs(x, Wqkv, bqkv, Wo, bo)
    res = run_bass_kernel_spmd(nc, in_maps, core_ids=list(range(NCORES)))
    outs = [res.results[c]["out"].reshape(BLOC, T, EMBED) for c in range(NCORES)]
    return np.concatenate(outs, axis=0)


if __name__ == "__main__":
    rng = np.random.default_rng(0)
    x = rng.standard_normal((B, T, EMBED)).astype(np.float32)
    Wqkv = (rng.random((EMBED, 3 * EMBED), np.float32) - 0.5) / 16.0
    bqkv = (rng.random(3 * EMBED).astype(np.float32) - 0.5) / 16.0
    Wo = (rng.random((EMBED, EMBED), np.float32) - 0.5) / 16.0
    bo = (rng.random(EMBED).astype(np.float32) - 0.5) / 16.0
    t0 = time.time()
    y = kernel(x=x, Wqkv=Wqkv, bqkv=bqkv, Wo=Wo, bo=bo)
    print("kernel done", time.time() - t0, y.shape, y.dtype)


# revision 48
# speedup vs baseline: 1.0182x; 1.0182x over previous
"""Multi-head causal attention (B=64, T=256, C=1024, H=16) on 8 TRN2 NeuronCores.

Data-parallel over batch (8 batches = 2048 tokens per core). V/out projections
and attention in bf16 (fp32 PSUM); the QK projection runs in fp8-e4m3
DoubleRow (2 MACs/cell/cycle, contraction 256 per matmul) — HW-measured at
the same 216ns issue spacing as a bf16 K=128 matmul, i.e. a true 2x.
~258 us HW vs the 318 us all-bf16 predecessor; output rel err 1.55e-2
(gate 2e-2, bf16 baseline was 3.7e-3).

Design notes:
  - QK fp8: x is quantized e4m3 at x*16 (host side, a second copy of x; the
    bf16 copy still feeds the V projection so V/attention accuracy is
    unaffected). W_q|W_k are quantized with a per-feature-column scale
    alpha_f = 120/max|col| to stay in e4m3's normal range (naive folding of
    1/sqrt(D) lands half the weights in fp8 subnormals). The DVE epilogue
    applies (psum * s_f + b_f) per feature partition via tensor_scalar,
    un-doing SX*alpha_f and folding the softmax scale + bias.
  - DoubleRow operands are 3D APs [128, 2, N]: plane i carries contraction
    rows 256*c2 + 128*i + p. Moving FD=512 (1024 fp8) is the ISA max; the
    256-col DR LDWEIGHTS (~135ns) hides under the chain.
  - Merged schedule: attention for batch b starts as soon as QK quarter b//2
    is done. Quarter 0 + 8 tiles of quarter 1 run in the prelude interleaved
    with V(0)/V(1); the rest drains one tile per attention pair through
    batches 0-5. This spreads the ACT/DVE softmax epilogue (~3.8us/pair)
    over the whole kernel span instead of a back-loaded attention phase.
  - Startup: input layouts are partition-major so every DMA piece is
    contiguous per partition — DMA enqueue time scales with descriptor
    count (a 1024-run strided piece measured 12us of engine time vs ~0.7us
    for 128 runs, stalling the issuing engine's FIFO and the PE). Pieces
    are split along consumption order over the sync/scalar/gpsimd rings
    (sync's ring measures ~62GB/s, the others ~125GB/s), late inputs are
    emitted mid-stream, and 20 warmup matmuls on a memset scratch region
    cover the ~7us framework preamble + DMA lead-in so the PE HAM
    clock-gate is at 8/8 when real work arrives.
  - Tail: the last batch's out-projection uses 4 PSUM banks (2 from ps_proj
    + 2 borrowed from ps_st, which is idle after the last s1): each chunk's
    ct 0..6 stream is emitted around the final s2 and the ct=7 matmul +
    bias-add + split DMAs of chunk i ride behind chunk i+1's stream, so
    output DMAs drain ~6us before the last matmul.
  - Attention per head pair (even head PSUM bank 0, odd bank 1): scores via
    2 matmuls over the causal block structure (s1 heads run concurrently in
    PE row groups 0:64/64:128 via auto tile_position); triangular mask added
    in PSUM by one DVE add per pair; exp on ACT; softmax denominator
    replicated across 64 partitions via v_aug ones columns so the
    reciprocal is one partition-parallel DVE op; normalize muls on GpSimd
    (no PSUM port, SBUF-only work). s2 runs both FD256 matmuls before the
    FD128s so the va LDWEIGHTS hides under the longer fills.
  - PSUM start=True resets the whole 2KB bank: only the first matmul of a
    chain sets it.
"""

import sys
import time

sys.path.insert(0, "/opt/trn_rl_repo")

import numpy as np
import ml_dtypes

BF16 = ml_dtypes.bfloat16
F8E4 = ml_dtypes.float8_e4m3

EMBED = 1024
HEADS = 16
DHEAD = 64
T = 256
B = 64
NCORES = 8
BLOC = B // NCORES          # 8 batches per core
TOK = BLOC * T              # 2048 tokens per core
CT = EMBED // 128           # 8 bf16 contraction tiles
C2 = 4                      # fp8 DoubleRow contraction tiles (256 each)
SX = 16.0                   # x fp8 pre-scale
ALPHA_TGT = 120.0           # per-column weight max after scaling

_CACHE = {}


def _build():
    import concourse.bacc as bacc
    import concourse.mybir as mybir
    import concourse.tile as tile

    f32 = mybir.dt.float32
    bf16 = mybir.dt.bfloat16
    f8 = mybir.dt.float8e4
    EXP = mybir.ActivationFunctionType.Exp
    DR = mybir.MatmulPerfMode.DoubleRow
    MUL = mybir.AluOpType.mult
    ADD = mybir.AluOpType.add

    nc = bacc.Bacc("TRN2", target_bir_lowering=False, debug=False)

    # all layouts are partition-major with whole DMA pieces contiguous per
    # partition: DMA enqueue (descriptor-generation) time scales with the
    # run count, and a 1024-run strided enqueue was measured at 12us of
    # engine time (vs ~0.7us for 128 contiguous runs)
    xt_d = nc.dram_tensor("xt", [128, 16, CT, 128], bf16, kind="ExternalInput").ap()
    x8_d = nc.dram_tensor("x8", [128, 4, C2, 2, 512], f8, kind="ExternalInput").ap()
    w8_d = nc.dram_tensor("w8", [128, 4, C2, 2, 512], f8, kind="ExternalInput").ap()
    wv_d = nc.dram_tensor("wv", [128, 2, 2, 4, 512], bf16, kind="ExternalInput").ap()
    wo_d = nc.dram_tensor("wo", [128, 2, 4, EMBED], bf16, kind="ExternalInput").ap()
    sqk_d = nc.dram_tensor("sqk", [128, 16, 2], f32, kind="ExternalInput").ap()
    bo2_d = nc.dram_tensor("bo2", [128, EMBED], bf16, kind="ExternalInput").ap()
    mtri_d = nc.dram_tensor("mtri", [128, 2, 2, 128], f32, kind="ExternalInput").ap()
    out_d = nc.dram_tensor("out", [TOK, EMBED], f32, kind="ExternalOutput").ap()

    with tile.TileContext(nc) as tc:
        with (
            tc.tile_pool(name="const", bufs=1) as constp,
            tc.tile_pool(name="expp", bufs=3) as expp,
            tc.tile_pool(name="recipp", bufs=2) as recipp,
            tc.tile_pool(name="ycpp", bufs=2) as ycpp,
            tc.tile_pool(name="ytp", bufs=2) as ytp,
            tc.tile_pool(name="outp", bufs=2) as outp,
            tc.tile_pool(name="ps_proj", bufs=2, space="PSUM") as ps_proj,
            tc.tile_pool(name="ps_st", bufs=2, space="PSUM") as ps_st,
            tc.tile_pool(name="ps_yt", bufs=2, space="PSUM") as ps_yt,
        ):
            xt_s = constp.tile([128, 16, CT, 128], bf16)   # [p, tpiece, ct, t']
            x8_s = constp.tile([128, 4, C2, 2, 512], f8)
            w8_s = constp.tile([128, 4, C2, 2, 512], f8)
            wv_s = constp.tile([128, 2, 2, 4, 512], bf16)  # [p, fjh, ch, ct', f']
            wo_s = constp.tile([128, 2, 4, EMBED], bf16)   # [p, ch, ct', f]
            sqk_s = constp.tile([128, 16, 2], f32)
            bo2_s = constp.tile([128, EMBED], bf16)
            mtri4_s = constp.tile([128, 2, 2, 128], f32)
            NVBUF = 2
            v_aug = constp.tile([128, NVBUF, 2, 16 * 128], bf16)
            ones_view = v_aug.rearrange("p u tt (h x) -> p u tt h x", x=128)
            qk_s = constp.tile([128, 16, TOK], bf16)

            # ---- PE warmup: memset a scratch region of qk_s (overwritten by
            # the QK epilogue much later), then 8 matmuls so HAM reaches 8/8
            # during the DMA lead-in ----
            junk = qk_s[:, 0, 0:640]
            nc.gpsimd.memset(junk, 1.0)
            wscr = ps_yt.tile([128, 2, T], f32, tag="yt_ps")
            wview = wscr.rearrange("p a b -> p (a b)")
            for i in range(20):
                nc.tensor.matmul(
                    wview[:, 0:512], junk[:, 0:128], junk[:, 128:640],
                    start=True, stop=True, skip_group_check=(i > 0),
                )

            # ---- input DMAs, split along consumption order, spread over
            # 4 queues (dependency granularity is per DMA instruction) ----

            # Upfront: ONLY the prelude's critical path (~5.3MB), all pieces
            # contiguous per partition. Everything else is emitted later in
            # the stream (staged below) so late transfers can't stall the
            # startup enqueues via ring/semaphore pressure.
            # Consumption order: warmup -> qk f0-3 -> V0(f0,tt0) ->
            # qk f4-7 -> V0(f0,tt1) -> qk f8-11 -> V0(f1,tt0) ->
            # qk f12-15 -> V0(f1,tt1) -> V1/qk-q1
            # (sync's DMA ring measures ~62GB/s vs ~125GB/s for the scalar/
            # gpsimd rings — keep sync's upfront load light)
            nc.sync.dma_start(out=sqk_s[:], in_=sqk_d[:])
            nc.sync.dma_start(out=xt_s[:, 0], in_=xt_d[:, 0])
            nc.sync.dma_start(out=xt_s[:, 1], in_=xt_d[:, 1])
            nc.sync.dma_start(out=w8_s[:, 3], in_=w8_d[:, 3])

            nc.scalar.dma_start(out=x8_s[:, 0], in_=x8_d[:, 0])
            nc.scalar.dma_start(out=wv_s[:, 0, 0], in_=wv_d[:, 0, 0])
            nc.scalar.dma_start(out=w8_s[:, 2], in_=w8_d[:, 2])
            nc.scalar.dma_start(out=wv_s[:, 1, 0], in_=wv_d[:, 1, 0])
            nc.scalar.dma_start(out=xt_s[:, 2], in_=xt_d[:, 2])
            nc.scalar.dma_start(out=xt_s[:, 3], in_=xt_d[:, 3])

            nc.gpsimd.dma_start(out=w8_s[:, 0], in_=w8_d[:, 0])
            nc.gpsimd.dma_start(out=wv_s[:, 0, 1], in_=wv_d[:, 0, 1])
            nc.gpsimd.memset(ones_view[:, 0, :, :, 0:64], 1.0)
            nc.gpsimd.dma_start(out=w8_s[:, 1], in_=w8_d[:, 1])
            nc.gpsimd.memset(ones_view[:, 1, :, :, 0:64], 1.0)
            nc.gpsimd.dma_start(out=wv_s[:, 1, 1], in_=wv_d[:, 1, 1])

            def xt_tok(ct, lo, n):
                """AP for xt tokens [lo, lo+n) of contraction chunk ct."""
                assert n == 128 and lo % 128 == 0
                return xt_s[:, lo // 128, ct, :]

            # ---------------- emission helpers ----------------

            def v_gen(b):
                """V projection of batch b (token-major into v_aug[b%2]).
                9 yields per chunk (chunk-atomic drains keep <=2 ps_proj
                tiles in flight)."""
                for fjh in range(2):
                    for tt in range(2):
                        ps = ps_proj.tile([128, 512], f32, tag="ps_proj")
                        tbase = b * T + tt * 128
                        for ct in range(CT):
                            nc.tensor.matmul(
                                ps[:],
                                xt_tok(ct, tbase, 128),
                                wv_s[:, fjh, ct // 4, ct % 4, :],
                                start=(ct == 0),
                                stop=(ct == CT - 1),
                            )
                            yield
                        dst = v_aug.rearrange("p u tt (h x) -> p u tt h x", x=128)[
                            :, b % NVBUF, tt, fjh * 8:(fjh + 1) * 8, 64:128
                        ]
                        nc.scalar.copy(
                            out=dst, in_=ps.rearrange("p (h x) -> p h x", x=64)
                        )
                        yield

            def qk_gen():
                """QK projection, fp8 DoubleRow, quarter-major. 5 yields
                per [128 feat x 512 tok] tile (4 MM + 1 DVE epilogue)."""
                for q in range(4):
                    for f in range(16):
                        piece, fo = divmod(f, 4)
                        ps = ps_proj.tile([128, 512], f32, tag="ps_proj")
                        for c2 in range(C2):
                            nc.tensor.matmul(
                                ps[:],
                                w8_s[:, piece, c2, :, fo * 128:(fo + 1) * 128],
                                x8_s[:, q, c2, :, :],
                                start=(c2 == 0),
                                stop=(c2 == C2 - 1),
                                perf_mode=DR,
                            )
                            yield
                        nc.vector.tensor_scalar(
                            out=qk_s[:, f, q * 512:(q + 1) * 512],
                            in0=ps[:],
                            scalar1=sqk_s[:, f, 0:1],
                            scalar2=sqk_s[:, f, 1:2],
                            op0=MUL,
                            op1=ADD,
                        )
                        yield

            def out_gen(b):
                """Output projection of batch b from yt tile. 9 yields/chunk."""
                yt = yt_tiles[b]
                engs = [nc.sync, nc.gpsimd]
                for tt in range(2):
                    for fjh in range(2):
                        ps = ps_proj.tile([128, 512], f32, tag="ps_proj")
                        tbase = b * T + tt * 128
                        for ct in range(CT):
                            nc.tensor.matmul(
                                ps[:],
                                yt[:, ct, tt * 128:(tt + 1) * 128],
                                wo_s[:, ct // 4, ct % 4,
                                     fjh * 512:(fjh + 1) * 512],
                                start=(ct == 0),
                                stop=(ct == CT - 1),
                            )
                            yield
                        o_sb = outp.tile([128, 512], f32, tag="o_sb")
                        nc.vector.tensor_add(
                            out=o_sb[:], in0=ps[:],
                            in1=bo2_s[:, fjh * 512:(fjh + 1) * 512],
                        )
                        engs[(2 * tt + fjh) % 2].dma_start(
                            out=out_d[tbase:tbase + 128,
                                      fjh * 512:(fjh + 1) * 512],
                            in_=o_sb[:],
                        )
                        yield

            def drain(gen, n):
                if gen is None:
                    return
                for _ in range(n):
                    if next(gen, StopIteration) is StopIteration:
                        return

            def attn_s1_pair(b, f):
                """Scores+exp for head pair (2f, 2f+1) of batch b.

                sT layout [128, hh, slot, 128]: per pair-half hh (even head in
                PSUM bank 0, odd in bank 1), slots = A (s-diag t0:128),
                B (s-diag t128:256), C (s0:128 strip t128:256), pad. The two
                hh halves run concurrently in PE row groups 0:64 / 64:128
                (tile_position auto-derived from base partitions).
                """
                t0 = b * T
                fq, fk = f, 8 + f
                sT = ps_st.tile([128, 2, 4, 128], f32, tag="sT")
                for hh in range(2):
                    po = hh * 64
                    qT = qk_s[po:po + 64, fq, t0:t0 + T]
                    kT = qk_s[po:po + 64, fk, t0:t0 + T]
                    nc.tensor.matmul(
                        sT[:, hh, 0::2, :], kT[:, 0:128], qT, start=True, stop=True
                    )
                    nc.tensor.matmul(
                        sT[:, hh, 1, :], kT[:, 128:256], qT[:, 128:256],
                        start=False, stop=True, skip_group_check=True,
                    )
                # triangular additive mask on all four diagonal blocks at once
                nc.vector.tensor_add(
                    out=sT[:, :, 0:2, :], in0=sT[:, :, 0:2, :], in1=mtri4_s[:]
                )
                expT = expp.tile([128, 2, 4, 128], bf16, tag="expT")
                nc.scalar.activation(
                    out=expT[:, :, 0:3, :], in_=sT[:, :, 0:3, :], func=EXP
                )
                return expT

            def attn_s2_pair(b, f, expT):
                yt = yt_tiles[b]
                va = v_aug[:, b % NVBUF]
                yp = ps_yt.tile([128, 2, T], f32, tag="yt_ps")
                # both FD256 matmuls first, FD128s after: the ~100ns va
                # LDWEIGHTS hides under a 107ns FD256 fill but not under a
                # 53ns FD128 fill, so alternating exposes it twice per pair
                for hh in range(2):
                    h = 2 * f + hh
                    nc.tensor.matmul(
                        yp[:, hh, :], va[:, 0, h * 128:(h + 1) * 128],
                        expT[:, hh, 0::2, :],
                        start=(hh == 0), stop=True, skip_group_check=(hh == 1),
                    )
                for hh in range(2):
                    h = 2 * f + hh
                    nc.tensor.matmul(
                        yp[:, hh, 128:256], va[:, 1, h * 128:(h + 1) * 128],
                        expT[:, hh, 1, :],
                        start=False, stop=True, skip_group_check=True,
                    )
                # rows 0:64 = denominators (replicated), 64:128 = unnormalized y
                rec = recipp.tile([64, 2, T], f32, tag="rec")
                nc.vector.reciprocal_approx_fast(out=rec[:], in_=yp[0:64, :, :])
                ycp = ycpp.tile([64, 2, T], f32, tag="ycp")
                nc.scalar.copy(out=ycp[:], in_=yp[64:128, :, :])
                for hh in range(2):
                    nc.gpsimd.tensor_mul(
                        out=yt[hh * 64:(hh + 1) * 64, f, :],
                        in0=ycp[:, hh, :], in1=rec[:, hh, :],
                    )

            # ---------------- schedule ----------------

            yt_tiles = {}

            # prelude: qk-tile quads and V(0) chunks interleaved in DMA
            # arrival order, then V(1) chunks with qk quarter-1 pairs.
            # Non-startup input DMAs are emitted in stages here, well before
            # their consumers but after the startup transfers.
            def junk_fill(n):
                """Dependency-free matmuls that absorb DMA-arrival jitter
                ahead of a consumer that may stall the in-order PE queue."""
                for _ in range(n):
                    nc.tensor.matmul(
                        wview[:, 0:512], junk[:, 0:128], junk[:, 128:640],
                        start=True, stop=True, skip_group_check=True,
                    )

            qkg = qk_gen()
            vg0 = v_gen(0)
            for i in range(4):
                if i >= 1:
                    junk_fill(3)
                drain(qkg, 20)
                if i >= 1:
                    junk_fill(2)
                drain(vg0, 9)
                if i == 0:
                    nc.sync.dma_start(out=mtri4_s[:], in_=mtri_d[:])
                    nc.scalar.dma_start(out=xt_s[:, 4:16], in_=xt_d[:, 4:16])
                    nc.gpsimd.dma_start(out=x8_s[:, 1], in_=x8_d[:, 1])
                if i == 2:
                    nc.sync.dma_start(out=bo2_s[:], in_=bo2_d[:])
                    nc.sync.dma_start(out=wo_s[:, 0], in_=wo_d[:, 0])
                    nc.gpsimd.dma_start(out=x8_s[:, 2], in_=x8_d[:, 2])
                    nc.gpsimd.dma_start(out=wo_s[:, 1], in_=wo_d[:, 1])
            nc.scalar.dma_start(out=x8_s[:, 3], in_=x8_d[:, 3])

            # attention, batch-major; QK quarters 1-3 drain one tile per
            # pair through b0-b5; V(b+1) / out(b-1) interleave as chunks.
            # pend (the s2 of the previous s1 pair) carries across batch
            # boundaries so the PE queue always has s1 work ahead of a
            # dependency-stalled s2.
            # b0 doubles as prelude phase 2: its s1/s2 pairs need only
            # quarter-0 data (resident early), so they keep the PE fed while
            # V(1), quarter-1 tiles, and the late xt/x8 pieces stream in.
            pend = None
            for b in range(BLOC):
                yt_tiles[b] = ytp.tile([128, CT, T], bf16, tag="yt", name=f"yt{b}")
                vg = v_gen(b + 1) if b <= 6 else None
                og = out_gen(b - 1) if b >= 1 else None
                for f in range(8):
                    e = attn_s1_pair(b, f)
                    if pend is not None:
                        attn_s2_pair(*pend)
                    drain(qkg, 10 if b == 0 else 5)
                    if vg is not None and f % 2 == 0:
                        drain(vg, 9)
                    if og is not None and f % 2 == 1:
                        drain(og, 9)
                    pend = (b, f, e)
                drain(vg, 99)
                drain(og, 99)
                if b >= 1:
                    del yt_tiles[b - 1]

            # tail: out-proj of the last batch on 4 PSUM banks (2 ps_proj +
            # 2 borrowed from ps_st, idle after the last s1). ct 0..6 of all
            # chunks surround the final s2 so only 4 ct=7 matmuls + bias
            # adds + split DMAs trail the last normalize.
            yt7 = yt_tiles[BLOC - 1]
            tail_ps = [ps_proj.tile([128, 512], f32, tag="ps_proj",
                                    name=f"tailp{i}")
                       for i in range(2)]
            for i in range(2):
                t = ps_st.tile([128, 2, 4, 128], f32, tag="sT",
                               name=f"tailb{i}")
                tail_ps.append(t.rearrange("p a b c -> p (a b c)")[:, 0:512])
            chunks = [(0, 0), (0, 1), (1, 0), (1, 1)]

            def tail_mm(ci, ct):
                tt, fjh = chunks[ci]
                nc.tensor.matmul(
                    tail_ps[ci][:],
                    yt7[:, ct, tt * 128:(tt + 1) * 128],
                    wo_s[:, ct // 4, ct % 4, fjh * 512:(fjh + 1) * 512],
                    start=(ct == 0), stop=(ct == CT - 1),
                )

            engs = [nc.sync, nc.scalar, nc.gpsimd]
            tb7 = (BLOC - 1) * T

            def tail_epi(ci):
                """ct=7 matmul + bias adds + split DMAs for tail chunk ci.
                The last chunk splits 4 ways so the final transfer drains in
                a quarter of the time."""
                tt, fjh = chunks[ci]
                tail_mm(ci, 7)
                o_sb = outp.tile([128, 512], f32, tag="o_sb",
                                 name=f"tailo{ci}")
                nh = 4 if ci == 3 else 2
                w = 512 // nh
                for hv in range(nh):
                    sl = slice(hv * w, (hv + 1) * w)
                    fl = fjh * 512 + hv * w
                    nc.vector.tensor_add(
                        out=o_sb[:, sl], in0=tail_ps[ci][:, sl],
                        in1=bo2_s[:, fl:fl + w],
                    )
                    engs[(2 * ci + hv) % 3].dma_start(
                        out=out_d[tb7 + tt * 128:tb7 + (tt + 1) * 128,
                                  fl:fl + w],
                        in_=o_sb[:, sl],
                    )

            # stagger: each chunk's ct7+epilogue is emitted behind the next
            # chunk's ct0-6 stream so DMAs start draining ~6us before the
            # last matmul instead of all trailing it
            for ct in range(7):
                tail_mm(0, ct)
            attn_s2_pair(*pend)
            for ct in range(7):
                tail_mm(1, ct)
            tail_epi(0)
            for ct in range(7):
                tail_mm(2, ct)
            tail_epi(1)
            for ct in range(7):
                tail_mm(3, ct)
            tail_epi(2)
            tail_epi(3)

    nc.compile()
    return nc


def _prep_inputs(x, Wqkv, bqkv, Wo, bo):
    """Host-side prep: shard batch, transpose x, fp8-quantize the QK path
    with per-feature-column weight scales, fold scales/biases, bf16."""
    scale = DHEAD ** -0.5
    Wq = Wqkv[:, :EMBED]
    Wk = Wqkv[:, EMBED:2 * EMBED]
    Wv = Wqkv[:, 2 * EMBED:]

    Wqk = np.concatenate([Wq, Wk], axis=1).astype(np.float32)   # [C, 2C]
    alpha = (ALPHA_TGT / np.abs(Wqk).max(axis=0)).astype(np.float32)
    w8 = (Wqk * alpha).astype(F8E4)
    # [k, f] -> [p, piece, c2, i, j]; k = c2*256 + i*128 + p, f = piece*512+j
    w8_f = np.ascontiguousarray(
        w8.reshape(C2, 2, 128, 4, 512).transpose(2, 3, 0, 1, 4)
    )

    mult = np.concatenate(
        [np.full(EMBED, scale, np.float32), np.ones(EMBED, np.float32)]
    ) / (SX * alpha)
    bias = np.concatenate(
        [bqkv[:EMBED] * scale, bqkv[EMBED:2 * EMBED]]
    ).astype(np.float32)
    sqk = np.stack([mult, bias], axis=-1)                        # [2C, 2]
    sqk_f = np.ascontiguousarray(sqk.reshape(16, 128, 2).transpose(1, 0, 2))

    # wv [p, fjh, ch, ct', f'] = Wv[(ch*4+ct')*128 + p, fjh*512 + f']
    wv_f = np.ascontiguousarray(
        Wv.astype(BF16).reshape(2, 4, 128, 2, 512).transpose(2, 3, 0, 1, 4)
    )
    # wo [p, ch, ct', f] = Wo[(ch*4+ct')*128 + p, f]
    wo_f = np.ascontiguousarray(
        Wo.astype(BF16).reshape(2, 4, 128, EMBED).transpose(2, 0, 1, 3)
    )

    bv = bqkv[2 * EMBED:]
    bo2 = (bv.astype(np.float64) @ Wo.astype(np.float64) + bo).astype(np.float32)
    bo2_f = np.ascontiguousarray(np.broadcast_to(bo2, (128, EMBED)).astype(BF16))

    s_idx = np.arange(128)[:, None]
    t_idx = np.arange(128)[None, :]
    mtri = np.where(s_idx <= t_idx, 0.0, -1e9).astype(np.float32)
    mtri = np.ascontiguousarray(
        np.broadcast_to(mtri[:, None, None, :], (128, 2, 2, 128))
    )

    in_maps = []
    for c in range(NCORES):
        xs = x[c * BLOC:(c + 1) * BLOC].reshape(TOK, EMBED)
        xT = np.ascontiguousarray(xs.T)                          # [C, TOK]
        # xt [p, tp, ct, t'] = xT[ct*128 + p, tp*128 + t']
        xt = np.ascontiguousarray(
            xT.astype(BF16).reshape(CT, 128, 16, 128).transpose(1, 2, 0, 3)
        )
        x8 = (xT * SX).astype(F8E4)
        # [k, t] -> [p, q, c2, i, j]; k = c2*256 + i*128 + p, t = q*512 + j
        x8_f = np.ascontiguousarray(
            x8.reshape(C2, 2, 128, 4, 512).transpose(2, 3, 0, 1, 4)
        )
        in_maps.append(
            {
                "xt": xt,
                "x8": x8_f,
                "w8": w8_f,
                "wv": wv_f,
                "wo": wo_f,
                "sqk": sqk_f,
                "bo2": bo2_f,
                "mtri": mtri,
            }
        )
    return in_maps


def kernel(x, Wqkv, bqkv, Wo, bo):
    from concourse.bass_utils import run_bass_kernel_spmd

    x = np.asarray(x, dtype=np.float32)
    Wqkv = np.asarray(Wqkv, dtype=np.float32)
    bqkv = np.asarray(bqkv, dtype=np.float32)
    Wo = np.asarray(Wo, dtype=np.float32)
    bo = np.asarray(bo, dtype=np.float32)

    if "nc" not in _CACHE:
        _CACHE["nc"] = _build()
    nc = _CACHE["nc"]

    in_maps = _prep_inputs(x, Wqkv, bqkv, Wo, bo)
    res = run_bass_kernel_spmd(nc, in_maps, core_ids=list(range(NCORES)))
    outs = [res.results[c]["out"].reshape(BLOC, T, EMBED) for c in range(NCORES)]
    return np.concatenate(outs, axis=0)


if __name__ == "__main__":
    rng = np.random.default_rng(0)
    x = rng.standard_normal((B, T, EMBED)).astype(np.float32)
    Wqkv = (rng.random((EMBED, 3 * EMBED), np.float32) - 0.5) / 16.0
    bqkv = (rng.random(3 * EMBED).astype(np.float32) - 0.5) / 16.0
    Wo = (rng.random((EMBED, EMBED), np.float32) - 0.5) / 16.0
    bo = (rng.random(EMBED).astype(np.float32) - 0.5) / 16.0
    t0 = time.time()
    y = kernel(x=x, Wqkv=Wqkv, bqkv=bqkv, Wo=Wo, bo=bo)
    print("kernel done", time.time() - t0, y.shape, y.dtype)


# revision 50
# speedup vs baseline: 1.0198x; 1.0016x over previous
"""Multi-head causal attention (B=64, T=256, C=1024, H=16) on 8 TRN2 NeuronCores.

Data-parallel over batch (8 batches = 2048 tokens per core). V/out projections
and attention in bf16 (fp32 PSUM); the QK projection runs in fp8-e4m3
DoubleRow (2 MACs/cell/cycle, contraction 256 per matmul) — HW-measured at
the same 216ns issue spacing as a bf16 K=128 matmul, i.e. a true 2x.
~258 us HW vs the 318 us all-bf16 predecessor; output rel err 1.55e-2
(gate 2e-2, bf16 baseline was 3.7e-3).

Design notes:
  - QK fp8: x is quantized e4m3 at x*16 (host side, a second copy of x; the
    bf16 copy still feeds the V projection so V/attention accuracy is
    unaffected). W_q|W_k are quantized with a per-feature-column scale
    alpha_f = 120/max|col| to stay in e4m3's normal range (naive folding of
    1/sqrt(D) lands half the weights in fp8 subnormals). The DVE epilogue
    applies (psum * s_f + b_f) per feature partition via tensor_scalar,
    un-doing SX*alpha_f and folding the softmax scale + bias.
  - DoubleRow operands are 3D APs [128, 2, N]: plane i carries contraction
    rows 256*c2 + 128*i + p. Moving FD=512 (1024 fp8) is the ISA max; the
    256-col DR LDWEIGHTS (~135ns) hides under the chain.
  - Merged schedule: attention for batch b starts as soon as QK quarter b//2
    is done. Quarter 0 + 8 tiles of quarter 1 run in the prelude interleaved
    with V(0)/V(1); the rest drains one tile per attention pair through
    batches 0-5. This spreads the ACT/DVE softmax epilogue (~3.8us/pair)
    over the whole kernel span instead of a back-loaded attention phase.
  - Startup: input layouts are partition-major so every DMA piece is
    contiguous per partition — DMA enqueue time scales with descriptor
    count (a 1024-run strided piece measured 12us of engine time vs ~0.7us
    for 128 runs, stalling the issuing engine's FIFO and the PE). Pieces
    are split along consumption order over the sync/scalar/gpsimd rings
    (sync's ring measures ~62GB/s, the others ~125GB/s), late inputs are
    emitted mid-stream, and 20 warmup matmuls on a memset scratch region
    cover the ~7us framework preamble + DMA lead-in so the PE HAM
    clock-gate is at 8/8 when real work arrives.
  - Tail: the last batch's out-projection uses 4 PSUM banks (2 from ps_proj
    + 2 borrowed from ps_st, which is idle after the last s1): each chunk's
    ct 0..6 stream is emitted around the final s2 and the ct=7 matmul +
    bias-add + split DMAs of chunk i ride behind chunk i+1's stream, so
    output DMAs drain ~6us before the last matmul.
  - Attention per head pair (even head PSUM bank 0, odd bank 1): scores via
    2 matmuls over the causal block structure (s1 heads run concurrently in
    PE row groups 0:64/64:128 via auto tile_position); triangular mask added
    in PSUM by one DVE add per pair; exp on ACT; softmax denominator
    replicated across 64 partitions via v_aug ones columns so the
    reciprocal is one partition-parallel DVE op; normalize muls on GpSimd
    (no PSUM port, SBUF-only work). s2 runs both FD256 matmuls before the
    FD128s so the va LDWEIGHTS hides under the longer fills.
  - PSUM start=True resets the whole 2KB bank: only the first matmul of a
    chain sets it.
"""

import sys
import time

sys.path.insert(0, "/opt/trn_rl_repo")

import numpy as np
import ml_dtypes

BF16 = ml_dtypes.bfloat16
F8E4 = ml_dtypes.float8_e4m3

EMBED = 1024
HEADS = 16
DHEAD = 64
T = 256
B = 64
NCORES = 8
BLOC = B // NCORES          # 8 batches per core
TOK = BLOC * T              # 2048 tokens per core
CT = EMBED // 128           # 8 bf16 contraction tiles
C2 = 4                      # fp8 DoubleRow contraction tiles (256 each)
SX = 16.0                   # x fp8 pre-scale
ALPHA_TGT = 120.0           # per-column weight max after scaling

_CACHE = {}


def _build():
    import concourse.bacc as bacc
    import concourse.mybir as mybir
    import concourse.tile as tile

    f32 = mybir.dt.float32
    bf16 = mybir.dt.bfloat16
    f8 = mybir.dt.float8e4
    EXP = mybir.ActivationFunctionType.Exp
    DR = mybir.MatmulPerfMode.DoubleRow
    MUL = mybir.AluOpType.mult
    ADD = mybir.AluOpType.add

    nc = bacc.Bacc("TRN2", target_bir_lowering=False, debug=False)

    # all layouts are partition-major with whole DMA pieces contiguous per
    # partition: DMA enqueue (descriptor-generation) time scales with the
    # run count, and a 1024-run strided enqueue was measured at 12us of
    # engine time (vs ~0.7us for 128 contiguous runs)
    xt_d = nc.dram_tensor("xt", [128, 16, CT, 128], bf16, kind="ExternalInput").ap()
    x8_d = nc.dram_tensor("x8", [128, 4, C2, 2, 512], f8, kind="ExternalInput").ap()
    w8_d = nc.dram_tensor("w8", [128, 4, C2, 2, 512], f8, kind="ExternalInput").ap()
    wv_d = nc.dram_tensor("wv", [128, 2, 2, 4, 512], bf16, kind="ExternalInput").ap()
    wo_d = nc.dram_tensor("wo", [128, 2, 4, EMBED], bf16, kind="ExternalInput").ap()
    sqk_d = nc.dram_tensor("sqk", [128, 16, 2], f32, kind="ExternalInput").ap()
    bo2_d = nc.dram_tensor("bo2", [128, EMBED], bf16, kind="ExternalInput").ap()
    mtri_d = nc.dram_tensor("mtri", [128, 2, 2, 128], f32, kind="ExternalInput").ap()
    out_d = nc.dram_tensor("out", [TOK, EMBED], f32, kind="ExternalOutput").ap()

    with tile.TileContext(nc) as tc:
        with (
            tc.tile_pool(name="const", bufs=1) as constp,
            tc.tile_pool(name="expp", bufs=3) as expp,
            tc.tile_pool(name="recipp", bufs=2) as recipp,
            tc.tile_pool(name="ycpp", bufs=2) as ycpp,
            tc.tile_pool(name="ytp", bufs=2) as ytp,
            tc.tile_pool(name="outp", bufs=2) as outp,
            tc.tile_pool(name="ps_proj", bufs=2, space="PSUM") as ps_proj,
            tc.tile_pool(name="ps_st", bufs=2, space="PSUM") as ps_st,
            tc.tile_pool(name="ps_yt", bufs=2, space="PSUM") as ps_yt,
        ):
            xt_s = constp.tile([128, 16, CT, 128], bf16)   # [p, tpiece, ct, t']
            x8_s = constp.tile([128, 4, C2, 2, 512], f8)
            w8_s = constp.tile([128, 4, C2, 2, 512], f8)
            wv_s = constp.tile([128, 2, 2, 4, 512], bf16)  # [p, fjh, ch, ct', f']
            wo_s = constp.tile([128, 2, 4, EMBED], bf16)   # [p, ch, ct', f]
            sqk_s = constp.tile([128, 16, 2], f32)
            bo2_s = constp.tile([128, EMBED], bf16)
            mtri4_s = constp.tile([128, 2, 2, 128], f32)
            NVBUF = 2
            v_aug = constp.tile([128, NVBUF, 2, 16 * 128], bf16)
            ones_view = v_aug.rearrange("p u tt (h x) -> p u tt h x", x=128)
            qk_s = constp.tile([128, 16, TOK], bf16)

            # ---- PE warmup: memset a scratch region of qk_s (overwritten by
            # the QK epilogue much later), then 8 matmuls so HAM reaches 8/8
            # during the DMA lead-in ----
            junk = qk_s[:, 0, 0:640]
            nc.gpsimd.memset(junk, 1.0)
            wscr = ps_yt.tile([128, 2, T], f32, tag="yt_ps")
            wview = wscr.rearrange("p a b -> p (a b)")
            for i in range(20):
                nc.tensor.matmul(
                    wview[:, 0:512], junk[:, 0:128], junk[:, 128:640],
                    start=True, stop=True, skip_group_check=(i > 0),
                )

            # ---- input DMAs, split along consumption order, spread over
            # 4 queues (dependency granularity is per DMA instruction) ----

            # Upfront: ONLY the prelude's critical path (~5.3MB), all pieces
            # contiguous per partition. Everything else is emitted later in
            # the stream (staged below) so late transfers can't stall the
            # startup enqueues via ring/semaphore pressure.
            # Consumption order: warmup -> qk f0-3 -> V0(f0,tt0) ->
            # qk f4-7 -> V0(f0,tt1) -> qk f8-11 -> V0(f1,tt0) ->
            # qk f12-15 -> V0(f1,tt1) -> V1/qk-q1
            # (sync's DMA ring measures ~62GB/s vs ~125GB/s for the scalar/
            # gpsimd rings — keep sync's upfront load light)
            nc.sync.dma_start(out=sqk_s[:], in_=sqk_d[:])
            nc.sync.dma_start(out=xt_s[:, 0], in_=xt_d[:, 0])
            nc.sync.dma_start(out=xt_s[:, 1], in_=xt_d[:, 1])
            nc.sync.dma_start(out=w8_s[:, 3], in_=w8_d[:, 3])

            nc.scalar.dma_start(out=x8_s[:, 0], in_=x8_d[:, 0])
            nc.scalar.dma_start(out=wv_s[:, 0, 0], in_=wv_d[:, 0, 0])
            nc.scalar.dma_start(out=w8_s[:, 2], in_=w8_d[:, 2])
            nc.scalar.dma_start(out=wv_s[:, 1, 0], in_=wv_d[:, 1, 0])
            nc.scalar.dma_start(out=xt_s[:, 2], in_=xt_d[:, 2])
            nc.scalar.dma_start(out=xt_s[:, 3], in_=xt_d[:, 3])

            nc.gpsimd.dma_start(out=w8_s[:, 0], in_=w8_d[:, 0])
            nc.gpsimd.dma_start(out=wv_s[:, 0, 1], in_=wv_d[:, 0, 1])
            nc.gpsimd.memset(ones_view[:, 0, :, :, 0:64], 1.0)
            nc.gpsimd.dma_start(out=w8_s[:, 1], in_=w8_d[:, 1])
            nc.gpsimd.memset(ones_view[:, 1, :, :, 0:64], 1.0)
            nc.gpsimd.dma_start(out=wv_s[:, 1, 1], in_=wv_d[:, 1, 1])

            def xt_tok(ct, lo, n):
                """AP for xt tokens [lo, lo+n) of contraction chunk ct."""
                assert n == 128 and lo % 128 == 0
                return xt_s[:, lo // 128, ct, :]

            # ---------------- emission helpers ----------------

            def v_gen(b):
                """V projection of batch b (token-major into v_aug[b%2]).
                9 yields per chunk (chunk-atomic drains keep <=2 ps_proj
                tiles in flight)."""
                for fjh in range(2):
                    for tt in range(2):
                        ps = ps_proj.tile([128, 512], f32, tag="ps_proj")
                        tbase = b * T + tt * 128
                        for ct in range(CT):
                            nc.tensor.matmul(
                                ps[:],
                                xt_tok(ct, tbase, 128),
                                wv_s[:, fjh, ct // 4, ct % 4, :],
                                start=(ct == 0),
                                stop=(ct == CT - 1),
                            )
                            yield
                        dst = v_aug.rearrange("p u tt (h x) -> p u tt h x", x=128)[
                            :, b % NVBUF, tt, fjh * 8:(fjh + 1) * 8, 64:128
                        ]
                        nc.scalar.copy(
                            out=dst, in_=ps.rearrange("p (h x) -> p h x", x=64)
                        )
                        yield

            def qk_gen():
                """QK projection, fp8 DoubleRow, quarter-major. 5 yields
                per [128 feat x 512 tok] tile (4 MM + 1 DVE epilogue)."""
                for q in range(4):
                    for f in range(16):
                        piece, fo = divmod(f, 4)
                        ps = ps_proj.tile([128, 512], f32, tag="ps_proj")
                        for c2 in range(C2):
                            nc.tensor.matmul(
                                ps[:],
                                w8_s[:, piece, c2, :, fo * 128:(fo + 1) * 128],
                                x8_s[:, q, c2, :, :],
                                start=(c2 == 0),
                                stop=(c2 == C2 - 1),
                                perf_mode=DR,
                            )
                            yield
                        nc.vector.tensor_scalar(
                            out=qk_s[:, f, q * 512:(q + 1) * 512],
                            in0=ps[:],
                            scalar1=sqk_s[:, f, 0:1],
                            scalar2=sqk_s[:, f, 1:2],
                            op0=MUL,
                            op1=ADD,
                        )
                        yield

            def out_gen(b):
                """Output projection of batch b from yt tile. 9 yields/chunk."""
                yt = yt_tiles[b]
                engs = [nc.sync, nc.gpsimd]
                for tt in range(2):
                    for fjh in range(2):
                        ps = ps_proj.tile([128, 512], f32, tag="ps_proj")
                        tbase = b * T + tt * 128
                        for ct in range(CT):
                            nc.tensor.matmul(
                                ps[:],
                                yt[:, ct, tt * 128:(tt + 1) * 128],
                                wo_s[:, ct // 4, ct % 4,
                                     fjh * 512:(fjh + 1) * 512],
                                start=(ct == 0),
                                stop=(ct == CT - 1),
                            )
                            yield
                        o_sb = outp.tile([128, 512], f32, tag="o_sb")
                        nc.vector.tensor_add(
                            out=o_sb[:], in0=ps[:],
                            in1=bo2_s[:, fjh * 512:(fjh + 1) * 512],
                        )
                        engs[(2 * tt + fjh) % 2].dma_start(
                            out=out_d[tbase:tbase + 128,
                                      fjh * 512:(fjh + 1) * 512],
                            in_=o_sb[:],
                        )
                        yield

            def drain(gen, n):
                if gen is None:
                    return
                for _ in range(n):
                    if next(gen, StopIteration) is StopIteration:
                        return

            def attn_s1_pair(b, f):
                """Scores+exp for head pair (2f, 2f+1) of batch b.

                sT layout [128, hh, slot, 128]: per pair-half hh (even head in
                PSUM bank 0, odd in bank 1), slots = A (s-diag t0:128),
                B (s-diag t128:256), C (s0:128 strip t128:256), pad. The two
                hh halves run concurrently in PE row groups 0:64 / 64:128
                (tile_position auto-derived from base partitions).
                """
                t0 = b * T
                fq, fk = f, 8 + f
                sT = ps_st.tile([128, 2, 4, 128], f32, tag="sT")
                for hh in range(2):
                    po = hh * 64
                    qT = qk_s[po:po + 64, fq, t0:t0 + T]
                    kT = qk_s[po:po + 64, fk, t0:t0 + T]
                    nc.tensor.matmul(
                        sT[:, hh, 0::2, :], kT[:, 0:128], qT, start=True, stop=True
                    )
                    nc.tensor.matmul(
                        sT[:, hh, 1, :], kT[:, 128:256], qT[:, 128:256],
                        start=False, stop=True, skip_group_check=True,
                    )
                # triangular additive mask on all four diagonal blocks at once
                nc.vector.tensor_add(
                    out=sT[:, :, 0:2, :], in0=sT[:, :, 0:2, :], in1=mtri4_s[:]
                )
                expT = expp.tile([128, 2, 4, 128], bf16, tag="expT")
                nc.scalar.activation(
                    out=expT[:, :, 0:3, :], in_=sT[:, :, 0:3, :], func=EXP
                )
                return expT

            def attn_s2_pair(b, f, expT):
                yt = yt_tiles[b]
                va = v_aug[:, b % NVBUF]
                yp = ps_yt.tile([128, 2, T], f32, tag="yt_ps")
                # both FD256 matmuls first, FD128s after: the ~100ns va
                # LDWEIGHTS hides under a 107ns FD256 fill but not under a
                # 53ns FD128 fill, so alternating exposes it twice per pair
                for hh in range(2):
                    h = 2 * f + hh
                    nc.tensor.matmul(
                        yp[:, hh, :], va[:, 0, h * 128:(h + 1) * 128],
                        expT[:, hh, 0::2, :],
                        start=(hh == 0), stop=True, skip_group_check=(hh == 1),
                    )
                for hh in range(2):
                    h = 2 * f + hh
                    nc.tensor.matmul(
                        yp[:, hh, 128:256], va[:, 1, h * 128:(h + 1) * 128],
                        expT[:, hh, 1, :],
                        start=False, stop=True, skip_group_check=True,
                    )
                # rows 0:64 = denominators (replicated), 64:128 = unnormalized y
                rec = recipp.tile([64, 2, T], f32, tag="rec")
                nc.vector.reciprocal_approx_fast(out=rec[:], in_=yp[0:64, :, :])
                ycp = ycpp.tile([64, 2, T], f32, tag="ycp")
                nc.scalar.copy(out=ycp[:], in_=yp[64:128, :, :])
                for hh in range(2):
                    nc.gpsimd.tensor_mul(
                        out=yt[hh * 64:(hh + 1) * 64, f, :],
                        in0=ycp[:, hh, :], in1=rec[:, hh, :],
                    )

            # ---------------- schedule ----------------

            yt_tiles = {}

            # prelude: qk-tile quads and V(0) chunks interleaved in DMA
            # arrival order, then V(1) chunks with qk quarter-1 pairs.
            # Non-startup input DMAs are emitted in stages here, well before
            # their consumers but after the startup transfers.
            def junk_fill(n):
                """Dependency-free matmuls that absorb DMA-arrival jitter
                ahead of a consumer that may stall the in-order PE queue."""
                for _ in range(n):
                    nc.tensor.matmul(
                        wview[:, 0:512], junk[:, 0:128], junk[:, 128:640],
                        start=True, stop=True, skip_group_check=True,
                    )

            qkg = qk_gen()
            vg0 = v_gen(0)
            for i in range(4):
                if i >= 1:
                    junk_fill(8 if i == 1 else 3)
                drain(qkg, 20)
                if i >= 1:
                    junk_fill(2)
                drain(vg0, 9)
                if i == 0:
                    nc.sync.dma_start(out=mtri4_s[:], in_=mtri_d[:])
                    nc.scalar.dma_start(out=xt_s[:, 4:16], in_=xt_d[:, 4:16])
                    nc.gpsimd.dma_start(out=x8_s[:, 1], in_=x8_d[:, 1])
                if i == 2:
                    nc.sync.dma_start(out=bo2_s[:], in_=bo2_d[:])
                    nc.sync.dma_start(out=wo_s[:, 0], in_=wo_d[:, 0])
                    nc.gpsimd.dma_start(out=x8_s[:, 2], in_=x8_d[:, 2])
                    nc.gpsimd.dma_start(out=wo_s[:, 1], in_=wo_d[:, 1])
            nc.scalar.dma_start(out=x8_s[:, 3], in_=x8_d[:, 3])

            # attention, batch-major; QK quarters 1-3 drain one tile per
            # pair through b0-b5; V(b+1) / out(b-1) interleave as chunks.
            # pend (the s2 of the previous s1 pair) carries across batch
            # boundaries so the PE queue always has s1 work ahead of a
            # dependency-stalled s2.
            # b0 doubles as prelude phase 2: its s1/s2 pairs need only
            # quarter-0 data (resident early), so they keep the PE fed while
            # V(1), quarter-1 tiles, and the late xt/x8 pieces stream in.
            pend = None
            for b in range(BLOC):
                yt_tiles[b] = ytp.tile([128, CT, T], bf16, tag="yt", name=f"yt{b}")
                vg = v_gen(b + 1) if b <= 6 else None
                og = out_gen(b - 1) if b >= 1 else None
                for f in range(8):
                    e = attn_s1_pair(b, f)
                    if pend is not None:
                        attn_s2_pair(*pend)
                    drain(qkg, 10 if b == 0 else 5)
                    if vg is not None and f % 2 == 0:
                        drain(vg, 9)
                    if og is not None and f % 2 == 1:
                        drain(og, 9)
                    pend = (b, f, e)
                drain(vg, 99)
                drain(og, 99)
                if b >= 1:
                    del yt_tiles[b - 1]

            # tail: out-proj of the last batch on 4 PSUM banks (2 ps_proj +
            # 2 borrowed from ps_st, idle after the last s1). ct 0..6 of all
            # chunks surround the final s2 so only 4 ct=7 matmuls + bias
            # adds + split DMAs trail the last normalize.
            yt7 = yt_tiles[BLOC - 1]
            tail_ps = [ps_proj.tile([128, 512], f32, tag="ps_proj",
                                    name=f"tailp{i}")
                       for i in range(2)]
            for i in range(2):
                t = ps_st.tile([128, 2, 4, 128], f32, tag="sT",
                               name=f"tailb{i}")
                tail_ps.append(t.rearrange("p a b c -> p (a b c)")[:, 0:512])
            chunks = [(0, 0), (0, 1), (1, 0), (1, 1)]

            def tail_mm(ci, ct):
                tt, fjh = chunks[ci]
                nc.tensor.matmul(
                    tail_ps[ci][:],
                    yt7[:, ct, tt * 128:(tt + 1) * 128],
                    wo_s[:, ct // 4, ct % 4, fjh * 512:(fjh + 1) * 512],
                    start=(ct == 0), stop=(ct == CT - 1),
                )

            engs = [nc.sync, nc.scalar, nc.gpsimd]
            tb7 = (BLOC - 1) * T

            def tail_epi(ci):
                """ct=7 matmul + bias adds + split DMAs for tail chunk ci.
                The last chunk splits 4 ways so the final transfer drains in
                a quarter of the time."""
                tt, fjh = chunks[ci]
                tail_mm(ci, 7)
                o_sb = outp.tile([128, 512], f32, tag="o_sb",
                                 name=f"tailo{ci}")
                nd = 2 if ci == 3 else 1   # DMA splits per add
                for hv in range(2):
                    sl = slice(hv * 256, (hv + 1) * 256)
                    fl = fjh * 512 + hv * 256
                    nc.vector.tensor_add(
                        out=o_sb[:, sl], in0=tail_ps[ci][:, sl],
                        in1=bo2_s[:, fl:fl + 256],
                    )
                    for dv in range(nd):
                        w = 256 // nd
                        f0 = fl + dv * w
                        engs[(2 * ci + 2 * hv + dv) % 3].dma_start(
                            out=out_d[tb7 + tt * 128:tb7 + (tt + 1) * 128,
                                      f0:f0 + w],
                            in_=o_sb[:, hv * 256 + dv * w:
                                     hv * 256 + (dv + 1) * w],
                        )

            # stagger: each chunk's ct7+epilogue is emitted behind the next
            # chunk's ct0-6 stream so DMAs start draining ~6us before the
            # last matmul instead of all trailing it
            for ct in range(7):
                tail_mm(0, ct)
            attn_s2_pair(*pend)
            for ct in range(7):
                tail_mm(1, ct)
            tail_epi(0)
            for ct in range(7):
                tail_mm(2, ct)
            tail_epi(1)
            for ct in range(7):
                tail_mm(3, ct)
            tail_epi(2)
            tail_epi(3)

    nc.compile()
    return nc


def _prep_inputs(x, Wqkv, bqkv, Wo, bo):
    """Host-side prep: shard batch, transpose x, fp8-quantize the QK path
    with per-feature-column weight scales, fold scales/biases, bf16."""
    scale = DHEAD ** -0.5
    Wq = Wqkv[:, :EMBED]
    Wk = Wqkv[:, EMBED:2 * EMBED]
    Wv = Wqkv[:, 2 * EMBED:]

    Wqk = np.concatenate([Wq, Wk], axis=1).astype(np.float32)   # [C, 2C]
    alpha = (ALPHA_TGT / np.abs(Wqk).max(axis=0)).astype(np.float32)
    w8 = (Wqk * alpha).astype(F8E4)
    # [k, f] -> [p, piece, c2, i, j]; k = c2*256 + i*128 + p, f = piece*512+j
    w8_f = np.ascontiguousarray(
        w8.reshape(C2, 2, 128, 4, 512).transpose(2, 3, 0, 1, 4)
    )

    mult = np.concatenate(
        [np.full(EMBED, scale, np.float32), np.ones(EMBED, np.float32)]
    ) / (SX * alpha)
    bias = np.concatenate(
        [bqkv[:EMBED] * scale, bqkv[EMBED:2 * EMBED]]
    ).astype(np.float32)
    sqk = np.stack([mult, bias], axis=-1)                        # [2C, 2]
    sqk_f = np.ascontiguousarray(sqk.reshape(16, 128, 2).transpose(1, 0, 2))

    # wv [p, fjh, ch, ct', f'] = Wv[(ch*4+ct')*128 + p, fjh*512 + f']
    wv_f = np.ascontiguousarray(
        Wv.astype(BF16).reshape(2, 4, 128, 2, 512).transpose(2, 3, 0, 1, 4)
    )
    # wo [p, ch, ct', f] = Wo[(ch*4+ct')*128 + p, f]
    wo_f = np.ascontiguousarray(
        Wo.astype(BF16).reshape(2, 4, 128, EMBED).transpose(2, 0, 1, 3)
    )

    bv = bqkv[2 * EMBED:]
    bo2 = (bv.astype(np.float64) @ Wo.astype(np.float64) + bo).astype(np.float32)
    bo2_f = np.ascontiguousarray(np.broadcast_to(bo2, (128, EMBED)).astype(BF16))

    s_idx = np.arange(128)[:, None]
    t_idx = np.arange(128)[None, :]
    mtri = np.where(s_idx <= t_idx, 0.0, -1e9).astype(np.float32)
    mtri = np.ascontiguousarray(
        np.broadcast_to(mtri[:, None, None, :], (128, 2, 2, 128))
    )

    in_maps = []
    for c in range(NCORES):
        xs = x[c * BLOC:(c + 1) * BLOC].reshape(TOK, EMBED)
        xT = np.ascontiguousarray(xs.T)                          # [C, TOK]
        # xt [p, tp, ct, t'] = xT[ct*128 + p, tp*128 + t']
        xt = np.ascontiguousarray(
            xT.astype(BF16).reshape(CT, 128, 16, 128).transpose(1, 2, 0, 3)
        )
        x8 = (xT * SX).astype(F8E4)
        # [k, t] -> [p, q, c2, i, j]; k = c2*256 + i*128 + p, t = q*512 + j
        x8_f = np.ascontiguousarray(
            x8.reshape(C2, 2, 128, 4, 512).transpose(2, 3, 0, 1, 4)
        )
        in_maps.append(
            {
                "xt": xt,
                "x8": x8_f,
                "w8": w8_f,
                "wv": wv_f,
                "wo": wo_f,
                "sqk": sqk_f,
                "bo2": bo2_f,
                "mtri": mtri,
            }
        )
    return in_maps


def kernel(x, Wqkv, bqkv, Wo, bo):
    from concourse.bass_utils import run_bass_kernel_spmd

    x = np.asarray(x, dtype=np.float32)
    Wqkv = np.asarray(Wqkv, dtype=np.float32)
    bqkv = np.asarray(bqkv, dtype=np.float32)
    Wo = np.asarray(Wo, dtype=np.float32)
    bo = np.asarray(bo, dtype=np.float32)

    if "nc" not in _CACHE:
        _CACHE["nc"] = _build()
    nc = _CACHE["nc"]

    in_maps = _prep_inputs(x, Wqkv, bqkv, Wo, bo)
    res = run_bass_kernel_spmd(nc, in_maps, core_ids=list(range(NCORES)))
    outs = [res.results[c]["out"].reshape(BLOC, T, EMBED) for c in range(NCORES)]
    return np.concatenate(outs, axis=0)


if __name__ == "__main__":
    rng = np.random.default_rng(0)
    x = rng.standard_normal((B, T, EMBED)).astype(np.float32)
    Wqkv = (rng.random((EMBED, 3 * EMBED), np.float32) - 0.5) / 16.0
    bqkv = (rng.random(3 * EMBED).astype(np.float32) - 0.5) / 16.0
    Wo = (rng.random((EMBED, EMBED), np.float32) - 0.5) / 16.0
    bo = (rng.random(EMBED).astype(np.float32) - 0.5) / 16.0
    t0 = time.time()
    y = kernel(x=x, Wqkv=Wqkv, bqkv=bqkv, Wo=Wo, bo=bo)
    print("kernel done", time.time() - t0, y.shape, y.dtype)
